# revision 47
# baseline (speedup 1.0000x reference)
"""Average Hausdorff loss on 8 Trainium2 NeuronCores.

Strategy
--------
Host (numpy, cheap): binarize masks, 3x3-erosion edge detection, compact
edge-pixel coordinates per (b, c) pair, build "augmented" coordinate
matrices so that a single K=6 bf16 matmul on the PE array produces the
exact value  -(squared distance)/4  for a [128 gth-pts, N pred-pts] tile
in PSUM (all products/partial sums are integers*0.25 < 2^24 -> exact
fp32; coords are centered so byte-split squared norms fit bf16 exactly).

Device (raw Bass, SPMD over 8 cores, 2 (b,c) pairs per core), pipelined
over PE -> ACT -> DVE per [128 gth x 1536 pred] chunk:
  PE : 3 matmuls -> PSUM = -(d^2)/4
  ACT: activation Copy with scale 2^-12 -> SBUF fp16 (sole PSUM reader)
  DVE: two fp16 2x halving folds + short reduce-max -> gth->pred NN,
       one fp16 2x tensor_max accumulate -> pred->gth NN
Host: final partition reduce for the pred->gth direction, sqrt, masked
means, nanmean -- tiny.

Pad points use a far sentinel coordinate so they never win a max.
"""

import numpy as np

H = 256
W = 256
BC = 16          # B*C pairs
N_CORES = 8
PAIRS_PER_CORE = 2
P_CHUNK = 1536   # pred points per DVE op (3 PSUM banks)
G_TILE = 128     # gth points per PE tile (PSUM partitions)
SENT = 16384.0   # sentinel coordinate (centered space), 2^14
D2_SCALE = 2.0 ** -12   # extra scale on -(d^2)/4 so fp16 never overflows
D2_BACK = -4.0 * 4096.0  # value -> d^2


def _edge_maps(x):
    """[BC, H, W] float -> bool edge maps, matching the reference:
    edge = mask & ~erode3x3(mask), erosion padded with True."""
    m = x > 0.5
    p = np.pad(m, ((0, 0), (1, 1), (1, 1)), constant_values=True)
    e = np.ones_like(m)
    for dy in range(3):
        for dx in range(3):
            e &= p[:, dy:dy + H, dx:dx + W]
    return m & ~e


def _compact_coords(edge):
    """bool [H, W] -> (cy, cx) float32 arrays of centered coords."""
    ys, xs = np.nonzero(edge)
    return (ys.astype(np.float32) - 128.0), (xs.astype(np.float32) - 128.0)


def _aug_g(cy, cx, n_pad):
    """lhsT rows [6, n_pad] for the stationary (gth) operand."""
    n = cy.shape[0]
    out = np.zeros((6, n_pad), np.float32)
    fy = np.full(n_pad, SENT, np.float32)
    fx = np.full(n_pad, SENT, np.float32)
    fy[:n] = cy
    fx[:n] = cx
    sq = fy * fy + fx * fx
    b1 = np.floor(sq / 256.0)
    b0 = sq - b1 * 256.0
    out[0] = fy * 0.5
    out[1] = fx * 0.5
    out[2] = -b1
    out[3] = -b0
    out[4] = -64.0
    out[5] = -0.25
    return out


def _aug_p(cy, cx, n_pad):
    """rhs rows [6, n_pad] for the moving (pred) operand."""
    n = cy.shape[0]
    out = np.zeros((6, n_pad), np.float32)
    fy = np.full(n_pad, SENT, np.float32)
    fx = np.full(n_pad, SENT, np.float32)
    fy[:n] = cy
    fx[:n] = cx
    sq = fy * fy + fx * fx
    b1 = np.floor(sq / 256.0)
    b0 = sq - b1 * 256.0
    out[0] = fy
    out[1] = fx
    out[2] = 64.0
    out[3] = 0.25
    out[4] = b1
    out[5] = b0
    return out


def _build_program(structure, self_waits=False):
    """structure: tuple of (n_gtiles, n_pchunks) per pair slot.

    Raw-bass program (no Tile): explicit semaphores, standalone waits.
    This walrus build rejects matmuls carrying >1 inline sync-wait, so
    the streams are arranged such that every instruction needs at most
    one cross-engine wait, emitted as its own EventSemaphore.

    self_waits adds same-engine DVE waits for RAW/WAR chains. Hardware
    orders these via the engine FIFO + per-op pipeline drain; the waits
    exist only to satisfy CoreSim's race detector (sim builds).
    """
    from contextlib import ExitStack
    import concourse.bass as bass
    import concourse.mybir as mybir

    f32 = mybir.dt.float32
    f16 = mybir.dt.float16
    bf16 = mybir.dt.bfloat16
    MAX = mybir.AluOpType.max

    nc = bass.Bass()

    gaug_d, paug_d, dg_d, dp_d = [], [], [], []
    for s, (tg, npc) in enumerate(structure):
        ng_pad = tg * G_TILE
        np_pad = npc * P_CHUNK
        gaug_d.append(nc.declare_dram_parameter(f"gaug{s}", [6, ng_pad], bf16,
                                                isOutput=False))
        paug_d.append(nc.declare_dram_parameter(f"paug{s}", [6, np_pad], bf16,
                                                isOutput=False))
        dg_d.append(nc.declare_dram_parameter(f"dg{s}", [G_TILE, tg], f32,
                                              isOutput=True))
        dp_d.append(nc.declare_dram_parameter(f"dp{s}", [G_TILE, np_pad], f16,
                                              isOutput=True))

    n_slots = len(structure)
    total_chunks = sum(tg * npc for tg, npc in structure)
    NB = 4  # d2s fp16 ring depth

    with ExitStack() as ctx:
        gs, ps, dp_acc, dg_st, dg_all = [], [], [], [], []
        for s, (tg, npc) in enumerate(structure):
            gs.append(ctx.enter_context(
                nc.sbuf_tensor(f"gs{s}", [6, tg * G_TILE], bf16)))
            ps.append(ctx.enter_context(
                nc.sbuf_tensor(f"ps{s}", [6, npc * P_CHUNK], bf16)))
            dp_acc.append(ctx.enter_context(
                nc.sbuf_tensor(f"dpacc{s}", [G_TILE, npc * P_CHUNK], f16)))
            dg_st.append(ctx.enter_context(
                nc.sbuf_tensor(f"dgst{s}", [G_TILE, tg, npc], f32)))
            dg_all.append(ctx.enter_context(
                nc.sbuf_tensor(f"dgall{s}", [G_TILE, tg], f32)))
        pt = [ctx.enter_context(nc.psum_tensor(f"pt{i}", [G_TILE, P_CHUNK], f32))
              for i in range(2)]
        # fp16 distance ring: 4 chunk slots in one tensor so adjacent pairs
        # (even k, odd k) can be consumed by single wide DVE ops.
        d2s = ctx.enter_context(
            nc.sbuf_tensor("d2s", [G_TILE, NB, P_CHUNK], f16))
        # fold buffers for the dg reduction (fp16 tt_max halving steps)
        fd1 = [ctx.enter_context(
            nc.sbuf_tensor(f"fd1_{i}", [G_TILE, 2, P_CHUNK // 2], f16))
            for i in range(2)]
        fd2 = [ctx.enter_context(
            nc.sbuf_tensor(f"fd2_{i}", [G_TILE, 2, P_CHUNK // 4], f16))
            for i in range(2)]
        fd3 = [ctx.enter_context(
            nc.sbuf_tensor(f"fd3_{i}", [G_TILE, P_CHUNK // 4], f16))
            for i in range(2)]
        fd4 = [ctx.enter_context(
            nc.sbuf_tensor(f"fd4_{i}", [G_TILE, P_CHUNK // 8], f16))
            for i in range(2)]

        dma_sems = [ctx.enter_context(nc.semaphore(f"dma_in{s}"))
                    for s in range(n_slots)]
        dma_ps = [ctx.enter_context(nc.semaphore(f"dma_ps{s}"))
                  for s in range(n_slots)]
        pe_sem = ctx.enter_context(nc.semaphore("pe_done"))
        act_sem = ctx.enter_context(nc.semaphore("act_done"))
        dve_sem = ctx.enter_context(nc.semaphore("dve_done"))
        out_sem = ctx.enter_context(nc.semaphore("dma_out"))
        block = ctx.enter_context(nc.Block())

        # Dry run of the DVE emission to get exact dve_sem values.
        # Groups: one per (slot, gt). npc==2 groups use paired (3072-wide)
        # DVE ops; other npc use per-chunk ops. 4 DVE incs per chunk-pair /
        # per chunk respectively; +1 final dg reduce per slot.
        chunk_last_read = []   # per chunk k: dve_sem when its d2s reads done
        slot_end = []
        _n = 0
        _k = 0
        for tg, npc in structure:
            paired = (npc == 2 and _k % 2 == 0)
            for gt in range(tg):
                if paired:
                    # flat group: 4 folds + reduce + dp max = 6 ops
                    _n += 6
                    chunk_last_read += [_n, _n]
                    _k += 2
                else:
                    for _ in range(npc):
                        _n += 4
                        chunk_last_read.append(_n)
                        _k += 1
            if not paired:
                _n += 1  # slot-final dg reduce (fallback path only)
            slot_end.append(_n)

        @block.sync
        def _(sync):
            for s in range(n_slots):
                sync.dma_start(gs[s][:], gaug_d[s][:]).then_inc(dma_sems[s], 16)
            for s in range(n_slots):
                sync.wait_ge(dve_sem, slot_end[s])
                sync.dma_start(dg_d[s][:], dg_all[s][:]).then_inc(out_sem, 16)
                sync.dma_start(dp_d[s][:], dp_acc[s][:]).then_inc(out_sem, 16)
            sync.wait_ge(out_sem, 16 * 2 * n_slots)

        @block.gpsimd
        def _(gpsimd):
            for s in range(n_slots):
                gpsimd.dma_start(ps[s][:], paug_d[s][:]).then_inc(dma_ps[s], 16)

        @block.tensor
        def _(tensor):
            k = 0
            for s, (tg, npc) in enumerate(structure):
                # start as soon as THIS slot's inputs have landed
                tensor.wait_ge(dma_sems[s], 16)
                tensor.wait_ge(dma_ps[s], 16)
                for gt in range(tg):
                    lhsT = gs[s][:, gt * G_TILE:(gt + 1) * G_TILE]
                    for pc in range(npc):
                        if k >= 2:
                            # psum slot reuse: ACT (sole PSUM reader) of
                            # chunk k-2 done
                            tensor.wait_ge(act_sem, k - 1)
                        p = pt[k % 2]
                        for b in range(P_CHUNK // 512):
                            off = pc * P_CHUNK + b * 512
                            mm = nc.tensor.matmul(
                                p[:, b * 512:(b + 1) * 512],
                                lhsT,
                                ps[s][:, off:off + 512],
                                start=True, stop=True,
                            )
                        mm.then_inc(pe_sem, 1)
                        k += 1

        @block.scalar
        def _(scalar):
            # PSUM fp32 -> SBUF fp16, scaled by 2^-12 so sentinel-pad
            # distances stay finite in fp16 (power-of-2: real values
            # keep their mantissa exactly).
            for k in range(total_chunks):
                scalar.wait_ge(pe_sem, k + 1)
                if k >= NB:
                    scalar.wait_ge(dve_sem, chunk_last_read[k - NB])
                nc.scalar.activation(
                    d2s[:, k % NB, :], pt[k % 2][:],
                    mybir.ActivationFunctionType.Copy, scale=D2_SCALE,
                ).then_inc(act_sem, 1)

        @block.vector
        def _(vector):
            H1 = P_CHUNK // 2
            H2 = P_CHUNK // 4
            k = 0
            n_ops = 0
            gi = 0            # group (gt) counter, for fold ring indexing
            writer = {}       # dp_acc region -> op count of its last write
            f_free = {}       # fold ring slot -> op count after its last read

            def dg_fold(din0, din1, f1, f1a, f1b, f2, out_col, ring):
                """fold-fold-reduce: d halves -> f1 -> f2 -> reduce."""
                nonlocal n_ops
                w = f_free.get(("f1", ring))
                if self_waits and w:
                    vector.wait_ge(dve_sem, w)  # f1 ring WAR
                nc.vector.tensor_max(f1, din0, din1).then_inc(dve_sem, 1)
                n_ops += 1
                w = f_free.get(("f2", ring))
                if self_waits:
                    vector.wait_ge(dve_sem, max(n_ops, w or 0))
                nc.vector.tensor_max(f2, f1a, f1b).then_inc(dve_sem, 1)
                n_ops += 1
                f_free[("f1", ring)] = n_ops
                if self_waits:
                    vector.wait_ge(dve_sem, n_ops)  # f2 RAW
                nc.vector.tensor_reduce(
                    out_col, f2, axis=mybir.AxisListType.X, op=MAX,
                ).then_inc(dve_sem, 1)
                n_ops += 1
                f_free[("f2", ring)] = n_ops

            def dp_accum(dpc, src, first):
                nonlocal n_ops
                if first:
                    ins = nc.vector.tensor_copy(dpc, src)
                else:
                    if self_waits:
                        vector.wait_ge(dve_sem, writer[id(dpc.tensor)])
                    ins = nc.vector.tensor_max(dpc, dpc, src)
                ins.then_inc(dve_sem, 1)
                n_ops += 1

            for s, (tg, npc) in enumerate(structure):
                paired = (npc == 2 and k % 2 == 0)
                for gt in range(tg):
                    r = gi % 2
                    if paired:
                        pr = k % NB  # even, pair occupies slots pr, pr+1
                        vector.wait_ge(act_sem, k + 2)
                        dpair = d2s[:, pr:pr + 2, :].rearrange("p a b -> p (a b)")
                        # flat fold chain over the whole 3072-wide group:
                        # each step halves at fp16 2x; tiny 1x reduce last.
                        chain = [
                            fd1[r][:].rearrange("p a b -> p (a b)"),
                            fd2[r][:].rearrange("p a b -> p (a b)"),
                            fd3[r][:],
                            fd4[r][:],
                        ]
                        src = dpair
                        W = 2 * P_CHUNK
                        for buf in chain:
                            if self_waits:
                                vector.wait_ge(dve_sem, n_ops)
                            nc.vector.tensor_max(
                                buf[:, 0:W // 2],
                                src[:, 0:W // 2], src[:, W // 2:W],
                            ).then_inc(dve_sem, 1)
                            n_ops += 1
                            src = buf
                            W //= 2
                        if self_waits:
                            vector.wait_ge(dve_sem, n_ops)
                        nc.vector.tensor_reduce(
                            dg_all[s][:, gt:gt + 1], src[:, 0:W],
                            axis=mybir.AxisListType.X, op=MAX,
                        ).then_inc(dve_sem, 1)
                        n_ops += 1
                        dpc = dp_acc[s][:, 0:2 * P_CHUNK]
                        dp_accum(dpc, dpair, gt == 0)
                        writer[id(dpc.tensor)] = n_ops
                        k += 2
                    else:
                        for pc in range(npc):
                            vector.wait_ge(act_sem, k + 1)
                            c = k % NB
                            f1 = fd1[r][:, 0, :]
                            f2 = fd2[r][:, 0, :]
                            dg_fold(
                                d2s[:, c, 0:H1], d2s[:, c, H1:P_CHUNK],
                                f1, f1[:, 0:H2], f1[:, H2:H1],
                                f2, dg_st[s][:, gt, pc:pc + 1], r,
                            )
                            dpc = dp_acc[s][:, pc * P_CHUNK:(pc + 1) * P_CHUNK]
                            dp_accum(dpc, d2s[:, c, :], gt == 0)
                            writer[id(dpc.tensor)] = n_ops
                            k += 1
                    gi += 1
                if not paired:
                    if self_waits:
                        vector.wait_ge(dve_sem, n_ops)  # dg_st writes done
                    nc.vector.tensor_reduce(
                        dg_all[s][:], dg_st[s][:],
                        axis=mybir.AxisListType.X, op=MAX,
                    ).then_inc(dve_sem, 1)
                    n_ops += 1

    return nc


def _loss_from_nn(dg_val, dp_val, n_g, n_p):
    """Mirror the reference combination. dg_val/dp_val are the device maxes
    of -(d^2)/4 * 2^-12 for the first n_g / n_p (valid) points."""
    with np.errstate(divide="ignore", invalid="ignore", over="ignore"):
        d_g = np.sqrt(np.maximum(D2_BACK * dg_val.astype(np.float64), 0.0))
        d_p = np.sqrt(np.maximum(D2_BACK * dp_val.astype(np.float64), 0.0))
        gth2pred = d_g.sum() / n_g if n_g > 0 else np.float64(np.nan)
        pred2gth = d_p.sum() / n_p if n_p > 0 else np.float64(np.nan)
        ahd = (gth2pred + pred2gth) / 2.0
        if n_g == 0 and n_p == 0:
            ahd = np.float64(np.nan)
        return 1.0 - 1.0 / (1.0 + ahd)


RUN_OPTS = {}    # extra kwargs for run_bass_kernel_spmd (test harness hook)
LAST_RES = None  # last BassKernelResults (test harness hook)


def kernel(gth, pred):
    from concourse.bass_utils import run_bass_kernel_spmd
    import ml_dtypes

    gth = np.asarray(gth, np.float32).reshape(BC, H, W)
    pred = np.asarray(pred, np.float32).reshape(BC, H, W)

    gedge = _edge_maps(gth)
    pedge = _edge_maps(pred)
    pts = []
    for i in range(BC):
        gy, gx = _compact_coords(gedge[i])
        py, px = _compact_coords(pedge[i])
        pts.append((gy, gx, py, px))

    # Balance pairs across cores: sort by tile cost, big+small per core.
    def cost(i):
        gy = pts[i][0]
        py = pts[i][2]
        return (max(1, -(-len(gy) // G_TILE)) * max(1, -(-len(py) // P_CHUNK)))
    order = sorted(range(BC), key=cost, reverse=True)
    assign = [[order[c], order[BC - 1 - c]] for c in range(N_CORES)]

    # Uniform per-slot structure = max over cores.
    structure = []
    for s in range(PAIRS_PER_CORE):
        tg = max(max(1, -(-len(pts[assign[c][s]][0]) // G_TILE))
                 for c in range(N_CORES))
        npc = max(max(1, -(-len(pts[assign[c][s]][2]) // P_CHUNK))
                  for c in range(N_CORES))
        structure.append((tg, npc))
    structure = tuple(structure)

    nc = _build_program(structure)

    in_maps = []
    for c in range(N_CORES):
        m = {}
        for s in range(PAIRS_PER_CORE):
            tg, npc = structure[s]
            gy, gx, py, px = pts[assign[c][s]]
            m[f"gaug{s}"] = _aug_g(gy, gx, tg * G_TILE).astype(ml_dtypes.bfloat16)
            m[f"paug{s}"] = _aug_p(py, px, npc * P_CHUNK).astype(ml_dtypes.bfloat16)
        in_maps.append(m)

    res = run_bass_kernel_spmd(nc, in_maps, list(range(N_CORES)), **RUN_OPTS)
    global LAST_RES
    LAST_RES = res
    results = res.results

    losses = np.full(BC, np.nan, np.float64)
    for c in range(N_CORES):
        for s in range(PAIRS_PER_CORE):
            i = assign[c][s]
            gy, gx, py, px = pts[i]
            n_g, n_p = len(gy), len(py)
            dg = np.asarray(results[c][f"dg{s}"], np.float64)   # [128, tg]
            dp = np.asarray(results[c][f"dp{s}"], np.float64)   # [128, np_pad]
            dg_flat = dg.T.reshape(-1)[:n_g]
            dp_red = dp.max(axis=0)[:n_p]
            losses[i] = _loss_from_nn(dg_flat, dp_red, n_g, n_p)

    return np.float32(np.nanmean(losses.astype(np.float32)))


# revision 48
# speedup vs baseline: 1.0045x; 1.0045x over previous
"""Average Hausdorff loss on 8 Trainium2 NeuronCores.

Strategy
--------
Host (numpy, cheap): binarize masks, 3x3-erosion edge detection, compact
edge-pixel coordinates per (b, c) pair, build "augmented" coordinate
matrices so that a single K=6 bf16 matmul on the PE array produces the
exact value  -(squared distance)/4  for a [128 gth-pts, N pred-pts] tile
in PSUM (all products/partial sums are integers*0.25 < 2^24 -> exact
fp32; coords are centered so byte-split squared norms fit bf16 exactly).

Device (raw Bass, SPMD over 8 cores, 2 (b,c) pairs per core), pipelined
over PE -> ACT -> DVE per [128 gth x 1536 pred] chunk:
  PE : 3 matmuls -> PSUM = -(d^2)/4
  ACT: activation Copy with scale 2^-12 -> SBUF fp16 (sole PSUM reader)
  DVE: two fp16 2x halving folds + short reduce-max -> gth->pred NN,
       one fp16 2x tensor_max accumulate -> pred->gth NN
Host: final partition reduce for the pred->gth direction, sqrt, masked
means, nanmean -- tiny.

Pad points use a far sentinel coordinate so they never win a max.
"""

import numpy as np

H = 256
W = 256
BC = 16          # B*C pairs
N_CORES = 8
PAIRS_PER_CORE = 2
P_CHUNK = 1536   # pred points per DVE op (3 PSUM banks)
G_TILE = 128     # gth points per PE tile (PSUM partitions)
SENT = 16384.0   # sentinel coordinate (centered space), 2^14
D2_SCALE = 2.0 ** -12   # extra scale on -(d^2)/4 so fp16 never overflows
D2_BACK = -4.0 * 4096.0  # value -> d^2


def _edge_maps(x):
    """[BC, H, W] float -> bool edge maps, matching the reference:
    edge = mask & ~erode3x3(mask), erosion padded with True."""
    m = x > 0.5
    p = np.pad(m, ((0, 0), (1, 1), (1, 1)), constant_values=True)
    e = np.ones_like(m)
    for dy in range(3):
        for dx in range(3):
            e &= p[:, dy:dy + H, dx:dx + W]
    return m & ~e


def _compact_coords(edge):
    """bool [H, W] -> (cy, cx) float32 arrays of centered coords."""
    ys, xs = np.nonzero(edge)
    return (ys.astype(np.float32) - 128.0), (xs.astype(np.float32) - 128.0)


def _aug_g(cy, cx, n_pad):
    """lhsT rows [6, n_pad] for the stationary (gth) operand."""
    n = cy.shape[0]
    out = np.zeros((6, n_pad), np.float32)
    fy = np.full(n_pad, SENT, np.float32)
    fx = np.full(n_pad, SENT, np.float32)
    fy[:n] = cy
    fx[:n] = cx
    sq = fy * fy + fx * fx
    b1 = np.floor(sq / 256.0)
    b0 = sq - b1 * 256.0
    out[0] = fy * 0.5
    out[1] = fx * 0.5
    out[2] = -b1
    out[3] = -b0
    out[4] = -64.0
    out[5] = -0.25
    return out


def _aug_p(cy, cx, n_pad):
    """rhs rows [6, n_pad] for the moving (pred) operand."""
    n = cy.shape[0]
    out = np.zeros((6, n_pad), np.float32)
    fy = np.full(n_pad, SENT, np.float32)
    fx = np.full(n_pad, SENT, np.float32)
    fy[:n] = cy
    fx[:n] = cx
    sq = fy * fy + fx * fx
    b1 = np.floor(sq / 256.0)
    b0 = sq - b1 * 256.0
    out[0] = fy
    out[1] = fx
    out[2] = 64.0
    out[3] = 0.25
    out[4] = b1
    out[5] = b0
    return out


def _build_program(structure, self_waits=False):
    """structure: tuple of (n_gtiles, n_pchunks) per pair slot.

    Raw-bass program (no Tile): explicit semaphores, standalone waits.
    This walrus build rejects matmuls carrying >1 inline sync-wait, so
    the streams are arranged such that every instruction needs at most
    one cross-engine wait, emitted as its own EventSemaphore.

    self_waits adds same-engine DVE waits for RAW/WAR chains. Hardware
    orders these via the engine FIFO + per-op pipeline drain; the waits
    exist only to satisfy CoreSim's race detector (sim builds).
    """
    from contextlib import ExitStack
    import concourse.bass as bass
    import concourse.mybir as mybir

    f32 = mybir.dt.float32
    f16 = mybir.dt.float16
    bf16 = mybir.dt.bfloat16
    MAX = mybir.AluOpType.max

    nc = bass.Bass()

    gaug_d, paug_d, dg_d, dp_d = [], [], [], []
    for s, (tg, npc) in enumerate(structure):
        ng_pad = tg * G_TILE
        np_pad = npc * P_CHUNK
        gaug_d.append(nc.declare_dram_parameter(f"gaug{s}", [6, ng_pad], bf16,
                                                isOutput=False))
        paug_d.append(nc.declare_dram_parameter(f"paug{s}", [6, np_pad], bf16,
                                                isOutput=False))
        dg_d.append(nc.declare_dram_parameter(f"dg{s}", [G_TILE, tg], f32,
                                              isOutput=True))
        dp_d.append(nc.declare_dram_parameter(f"dp{s}", [G_TILE, np_pad], f16,
                                              isOutput=True))

    n_slots = len(structure)
    total_chunks = sum(tg * npc for tg, npc in structure)
    NB = 4  # d2s fp16 ring depth

    with ExitStack() as ctx:
        gs, ps, dp_acc, dg_st, dg_all = [], [], [], [], []
        for s, (tg, npc) in enumerate(structure):
            gs.append(ctx.enter_context(
                nc.sbuf_tensor(f"gs{s}", [6, tg * G_TILE], bf16)))
            ps.append(ctx.enter_context(
                nc.sbuf_tensor(f"ps{s}", [6, npc * P_CHUNK], bf16)))
            dp_acc.append(ctx.enter_context(
                nc.sbuf_tensor(f"dpacc{s}", [G_TILE, npc * P_CHUNK], f16)))
            dg_st.append(ctx.enter_context(
                nc.sbuf_tensor(f"dgst{s}", [G_TILE, tg, npc], f32)))
            dg_all.append(ctx.enter_context(
                nc.sbuf_tensor(f"dgall{s}", [G_TILE, tg], f32)))
        pt = [ctx.enter_context(nc.psum_tensor(f"pt{i}", [G_TILE, P_CHUNK], f32))
              for i in range(2)]
        # fp16 distance ring: 4 chunk slots in one tensor so adjacent pairs
        # (even k, odd k) can be consumed by single wide DVE ops.
        d2s = ctx.enter_context(
            nc.sbuf_tensor("d2s", [G_TILE, NB, P_CHUNK], f16))
        # fold buffers for the dg reduction (fp16 tt_max halving steps)
        fd1 = [ctx.enter_context(
            nc.sbuf_tensor(f"fd1_{i}", [G_TILE, 2, P_CHUNK // 2], f16))
            for i in range(2)]
        fd2 = [ctx.enter_context(
            nc.sbuf_tensor(f"fd2_{i}", [G_TILE, 2, P_CHUNK // 4], f16))
            for i in range(2)]
        fd3 = [ctx.enter_context(
            nc.sbuf_tensor(f"fd3_{i}", [G_TILE, P_CHUNK // 4], f16))
            for i in range(2)]
        fd4 = [ctx.enter_context(
            nc.sbuf_tensor(f"fd4_{i}", [G_TILE, P_CHUNK // 8], f16))
            for i in range(2)]

        dma_sems = [ctx.enter_context(nc.semaphore(f"dma_in{s}"))
                    for s in range(n_slots)]
        pe_sem = ctx.enter_context(nc.semaphore("pe_done"))
        act_sem = ctx.enter_context(nc.semaphore("act_done"))
        dve_sem = ctx.enter_context(nc.semaphore("dve_done"))
        out_sem = ctx.enter_context(nc.semaphore("dma_out"))
        block = ctx.enter_context(nc.Block())

        # Dry run of the DVE emission to get exact dve_sem values.
        # Groups: one per (slot, gt). npc==2 groups use paired (3072-wide)
        # DVE ops; other npc use per-chunk ops. 4 DVE incs per chunk-pair /
        # per chunk respectively; +1 final dg reduce per slot.
        chunk_last_read = []   # per chunk k: dve_sem when its d2s reads done
        slot_end = []
        _n = 0
        _k = 0
        for tg, npc in structure:
            paired = (npc == 2 and _k % 2 == 0)
            for gt in range(tg):
                if paired:
                    # flat group: 4 folds + reduce + dp max = 6 ops
                    _n += 6
                    chunk_last_read += [_n, _n]
                    _k += 2
                else:
                    for _ in range(npc):
                        _n += 4
                        chunk_last_read.append(_n)
                        _k += 1
            if not paired:
                _n += 1  # slot-final dg reduce (fallback path only)
            slot_end.append(_n)

        @block.sync
        def _(sync):
            for s in range(n_slots):
                sync.dma_start(gs[s][:], gaug_d[s][:]).then_inc(dma_sems[s], 16)
                sync.dma_start(ps[s][:], paug_d[s][:]).then_inc(dma_sems[s], 16)
            for s in range(n_slots):
                sync.wait_ge(dve_sem, slot_end[s])
                sync.dma_start(dg_d[s][:], dg_all[s][:]).then_inc(out_sem, 16)
                sync.dma_start(dp_d[s][:], dp_acc[s][:]).then_inc(out_sem, 16)
            sync.wait_ge(out_sem, 16 * 2 * n_slots)

        @block.tensor
        def _(tensor):
            k = 0
            for s, (tg, npc) in enumerate(structure):
                # start as soon as THIS slot's inputs have landed
                tensor.wait_ge(dma_sems[s], 32)
                for gt in range(tg):
                    lhsT = gs[s][:, gt * G_TILE:(gt + 1) * G_TILE]
                    for pc in range(npc):
                        if k >= 2:
                            # psum slot reuse: ACT (sole PSUM reader) of
                            # chunk k-2 done
                            tensor.wait_ge(act_sem, k - 1)
                        p = pt[k % 2]
                        for b in range(P_CHUNK // 512):
                            off = pc * P_CHUNK + b * 512
                            mm = nc.tensor.matmul(
                                p[:, b * 512:(b + 1) * 512],
                                lhsT,
                                ps[s][:, off:off + 512],
                                start=True, stop=True,
                            )
                        mm.then_inc(pe_sem, 1)
                        k += 1

        @block.scalar
        def _(scalar):
            # PSUM fp32 -> SBUF fp16, scaled by 2^-12 so sentinel-pad
            # distances stay finite in fp16 (power-of-2: real values
            # keep their mantissa exactly).
            for k in range(total_chunks):
                scalar.wait_ge(pe_sem, k + 1)
                if k >= NB:
                    scalar.wait_ge(dve_sem, chunk_last_read[k - NB])
                nc.scalar.activation(
                    d2s[:, k % NB, :], pt[k % 2][:],
                    mybir.ActivationFunctionType.Copy, scale=D2_SCALE,
                ).then_inc(act_sem, 1)

        @block.vector
        def _(vector):
            H1 = P_CHUNK // 2
            H2 = P_CHUNK // 4
            k = 0
            n_ops = 0
            gi = 0            # group (gt) counter, for fold ring indexing
            writer = {}       # dp_acc region -> op count of its last write
            f_free = {}       # fold ring slot -> op count after its last read

            def dg_fold(din0, din1, f1, f1a, f1b, f2, out_col, ring):
                """fold-fold-reduce: d halves -> f1 -> f2 -> reduce."""
                nonlocal n_ops
                w = f_free.get(("f1", ring))
                if self_waits and w:
                    vector.wait_ge(dve_sem, w)  # f1 ring WAR
                nc.vector.tensor_max(f1, din0, din1).then_inc(dve_sem, 1)
                n_ops += 1
                w = f_free.get(("f2", ring))
                if self_waits:
                    vector.wait_ge(dve_sem, max(n_ops, w or 0))
                nc.vector.tensor_max(f2, f1a, f1b).then_inc(dve_sem, 1)
                n_ops += 1
                f_free[("f1", ring)] = n_ops
                if self_waits:
                    vector.wait_ge(dve_sem, n_ops)  # f2 RAW
                nc.vector.tensor_reduce(
                    out_col, f2, axis=mybir.AxisListType.X, op=MAX,
                ).then_inc(dve_sem, 1)
                n_ops += 1
                f_free[("f2", ring)] = n_ops

            def dp_accum(dpc, src, first):
                nonlocal n_ops
                if first:
                    ins = nc.vector.tensor_copy(dpc, src)
                else:
                    if self_waits:
                        vector.wait_ge(dve_sem, writer[id(dpc.tensor)])
                    ins = nc.vector.tensor_max(dpc, dpc, src)
                ins.then_inc(dve_sem, 1)
                n_ops += 1

            for s, (tg, npc) in enumerate(structure):
                paired = (npc == 2 and k % 2 == 0)
                for gt in range(tg):
                    r = gi % 2
                    if paired:
                        pr = k % NB  # even, pair occupies slots pr, pr+1
                        vector.wait_ge(act_sem, k + 2)
                        dpair = d2s[:, pr:pr + 2, :].rearrange("p a b -> p (a b)")
                        # flat fold chain over the whole 3072-wide group:
                        # each step halves at fp16 2x; tiny 1x reduce last.
                        chain = [
                            fd1[r][:].rearrange("p a b -> p (a b)"),
                            fd2[r][:].rearrange("p a b -> p (a b)"),
                            fd3[r][:],
                            fd4[r][:],
                        ]
                        src = dpair
                        W = 2 * P_CHUNK
                        for buf in chain:
                            if self_waits:
                                vector.wait_ge(dve_sem, n_ops)
                            nc.vector.tensor_max(
                                buf[:, 0:W // 2],
                                src[:, 0:W // 2], src[:, W // 2:W],
                            ).then_inc(dve_sem, 1)
                            n_ops += 1
                            src = buf
                            W //= 2
                        if self_waits:
                            vector.wait_ge(dve_sem, n_ops)
                        nc.vector.tensor_reduce(
                            dg_all[s][:, gt:gt + 1], src[:, 0:W],
                            axis=mybir.AxisListType.X, op=MAX,
                        ).then_inc(dve_sem, 1)
                        n_ops += 1
                        dpc = dp_acc[s][:, 0:2 * P_CHUNK]
                        dp_accum(dpc, dpair, gt == 0)
                        writer[id(dpc.tensor)] = n_ops
                        k += 2
                    else:
                        for pc in range(npc):
                            vector.wait_ge(act_sem, k + 1)
                            c = k % NB
                            f1 = fd1[r][:, 0, :]
                            f2 = fd2[r][:, 0, :]
                            dg_fold(
                                d2s[:, c, 0:H1], d2s[:, c, H1:P_CHUNK],
                                f1, f1[:, 0:H2], f1[:, H2:H1],
                                f2, dg_st[s][:, gt, pc:pc + 1], r,
                            )
                            dpc = dp_acc[s][:, pc * P_CHUNK:(pc + 1) * P_CHUNK]
                            dp_accum(dpc, d2s[:, c, :], gt == 0)
                            writer[id(dpc.tensor)] = n_ops
                            k += 1
                    gi += 1
                if not paired:
                    if self_waits:
                        vector.wait_ge(dve_sem, n_ops)  # dg_st writes done
                    nc.vector.tensor_reduce(
                        dg_all[s][:], dg_st[s][:],
                        axis=mybir.AxisListType.X, op=MAX,
                    ).then_inc(dve_sem, 1)
                    n_ops += 1

    return nc


def _loss_from_nn(dg_val, dp_val, n_g, n_p):
    """Mirror the reference combination. dg_val/dp_val are the device maxes
    of -(d^2)/4 * 2^-12 for the first n_g / n_p (valid) points."""
    with np.errstate(divide="ignore", invalid="ignore", over="ignore"):
        d_g = np.sqrt(np.maximum(D2_BACK * dg_val.astype(np.float64), 0.0))
        d_p = np.sqrt(np.maximum(D2_BACK * dp_val.astype(np.float64), 0.0))
        gth2pred = d_g.sum() / n_g if n_g > 0 else np.float64(np.nan)
        pred2gth = d_p.sum() / n_p if n_p > 0 else np.float64(np.nan)
        ahd = (gth2pred + pred2gth) / 2.0
        if n_g == 0 and n_p == 0:
            ahd = np.float64(np.nan)
        return 1.0 - 1.0 / (1.0 + ahd)


RUN_OPTS = {}    # extra kwargs for run_bass_kernel_spmd (test harness hook)
LAST_RES = None  # last BassKernelResults (test harness hook)


def kernel(gth, pred):
    from concourse.bass_utils import run_bass_kernel_spmd
    import ml_dtypes

    gth = np.asarray(gth, np.float32).reshape(BC, H, W)
    pred = np.asarray(pred, np.float32).reshape(BC, H, W)

    gedge = _edge_maps(gth)
    pedge = _edge_maps(pred)
    pts = []
    for i in range(BC):
        gy, gx = _compact_coords(gedge[i])
        py, px = _compact_coords(pedge[i])
        pts.append((gy, gx, py, px))

    # Balance pairs across cores: sort by tile cost, big+small per core.
    def cost(i):
        gy = pts[i][0]
        py = pts[i][2]
        return (max(1, -(-len(gy) // G_TILE)) * max(1, -(-len(py) // P_CHUNK)))
    order = sorted(range(BC), key=cost, reverse=True)
    assign = [[order[c], order[BC - 1 - c]] for c in range(N_CORES)]

    # Uniform per-slot structure = max over cores.
    structure = []
    for s in range(PAIRS_PER_CORE):
        tg = max(max(1, -(-len(pts[assign[c][s]][0]) // G_TILE))
                 for c in range(N_CORES))
        npc = max(max(1, -(-len(pts[assign[c][s]][2]) // P_CHUNK))
                  for c in range(N_CORES))
        structure.append((tg, npc))
    structure = tuple(structure)

    nc = _build_program(structure)

    in_maps = []
    for c in range(N_CORES):
        m = {}
        for s in range(PAIRS_PER_CORE):
            tg, npc = structure[s]
            gy, gx, py, px = pts[assign[c][s]]
            m[f"gaug{s}"] = _aug_g(gy, gx, tg * G_TILE).astype(ml_dtypes.bfloat16)
            m[f"paug{s}"] = _aug_p(py, px, npc * P_CHUNK).astype(ml_dtypes.bfloat16)
        in_maps.append(m)

    res = run_bass_kernel_spmd(nc, in_maps, list(range(N_CORES)), **RUN_OPTS)
    global LAST_RES
    LAST_RES = res
    results = res.results

    losses = np.full(BC, np.nan, np.float64)
    for c in range(N_CORES):
        for s in range(PAIRS_PER_CORE):
            i = assign[c][s]
            gy, gx, py, px = pts[i]
            n_g, n_p = len(gy), len(py)
            dg = np.asarray(results[c][f"dg{s}"], np.float64)   # [128, tg]
            dp = np.asarray(results[c][f"dp{s}"], np.float64)   # [128, np_pad]
            dg_flat = dg.T.reshape(-1)[:n_g]
            dp_red = dp.max(axis=0)[:n_p]
            losses[i] = _loss_from_nn(dg_flat, dp_red, n_g, n_p)

    return np.float32(np.nanmean(losses.astype(np.float32)))
